# revision 1
# baseline (speedup 1.0000x reference)
"""GCGRU cell (order-2 graph diffusion GRU) Trainium2 Bass kernel.

Strategy: data-parallel over batch (B=16 -> 2 batches per core x 8 cores).
Per core, activations are kept node-major ([node-chunk partitions x (b,c)
columns], fp16) so the graph-diffusion matmuls (contract over the node dim)
run with adj^T tiles as the PE-stationary operand, streamed from HBM and
accumulated over n-chunks in PSUM. The node dim is zero-padded to 4096 so
every tile is a full 128 partitions / 128 columns (fast weight load). adj^T
is host-retiled partition-major so each slab DMA is one dense transfer with
multi-KB contiguous runs per partition.

The gates share one diffusion of z=[x;h]; since z1=A z already contains A x,
the candidate path only diffuses r*h (128 batch-channel columns), using r*h as
the PE-stationary operand and adj as the 512-wide moving operand, producing
batch-major outputs that feed the candidate conv directly. The final candidate
conv + tanh + u*h+(1-u)*c combine is fused into the last diffusion's PSUM
group loop so the kernel tail is one group deep. sigmoid/tanh on ScalarE.
All input casts/layout transforms are done on host in kernel().
"""

import numpy as np

import concourse.bass as bass
from concourse import bacc
import concourse.mybir as mybir
import concourse.tile as tile
from concourse.bass_utils import run_bass_kernel_spmd

# problem constants
B, D_IN, D_H, NN = 16, 32, 64, 4000
NCORES = 8
B_LOC = B // NCORES          # batches per core
C = D_IN + D_H               # 96 channels into each gate conv
BC = B_LOC * C               # node-major column count (b-major: [b0 c96 | b1 c96])
BH = B_LOC * D_H             # stacked batch-hidden rows (128)
NP = 4096                    # node dim padded to a multiple of 128

F16 = mybir.dt.float16
F32 = mybir.dt.float32
CHUNK = 128


def build_program(npad=NP, nn=NN, mg=4, jb=8, nsl=512):
    """Build the single-core Bass program (same program runs SPMD on 8 cores).

    npad: padded node count; mg: m-chunks per PSUM group; jb: n-chunk blocks
    merged per slab DMA; nsl: node slice width for conv/elementwise loops.
    """
    chunk = CHUNK
    nch = npad // chunk          # node chunks
    ngrp = nch // mg             # psum groups per diffusion stage
    nsli = npad // nsl           # conv node slices
    assert nch % mg == 0 and npad % nsl == 0 and nch % jb == 0
    assert nsl == mg * chunk     # fused consumer: conv slice == psum group band
    assert BH == chunk

    nc = bacc.Bacc("TRN2", target_bir_lowering=False, debug=False)

    # ---- DRAM I/O (all host-prepped layouts) ----
    # at_t[g, p, j, :] = adjT[j*128+p, g*mg*128:(g+1)*mg*128]  (partition-major:
    # per partition, all n-chunk blocks of a group band are contiguous)
    at_d = nc.dram_tensor("at", [ngrp, chunk, nch, mg * chunk], F16,
                          kind="ExternalInput").ap()
    zt_d = nc.dram_tensor("zt", [npad, BC], F16, kind="ExternalInput").ap()
    xh_d = nc.dram_tensor("xh", [B_LOC, C, npad], F16, kind="ExternalInput").ap()
    h_d = nc.dram_tensor("h", [B_LOC, D_H, npad], F16, kind="ExternalInput").ap()
    wf_d = nc.dram_tensor("wf", [3, C, D_H], F16, kind="ExternalInput").ap()
    wu_d = nc.dram_tensor("wu", [3, C, D_H], F16, kind="ExternalInput").ap()
    # candidate weights: x rows per diffusion order, and batch-duplicated rh rows
    wcx_d = nc.dram_tensor("wcx", [3, D_IN, D_H], F16, kind="ExternalInput").ap()
    wcrh_d = nc.dram_tensor("wcrh", [3, BH, D_H], F16, kind="ExternalInput").ap()
    bf_d = nc.dram_tensor("bf", [BH, 1], F32, kind="ExternalInput").ap()
    bu_d = nc.dram_tensor("bu", [BH, 1], F32, kind="ExternalInput").ap()
    bc_d = nc.dram_tensor("bcb", [BH, 1], F32, kind="ExternalInput").ap()
    id_d = nc.dram_tensor("idm", [chunk, chunk], F16, kind="ExternalInput").ap()
    out_d = nc.dram_tensor("out", [B_LOC, D_H, nn], F32, kind="ExternalOutput").ap()

    with tile.TileContext(nc) as tc:
        _body(tc, locals())
    nc.compile()
    return nc


def _body(tc, aps):
    nc = tc.nc
    npad, nn, chunk, mg, jb, nsl = (aps[k] for k in
                                    ("npad", "nn", "chunk", "mg", "jb", "nsl"))
    nch, ngrp, nsli = aps["nch"], aps["ngrp"], aps["nsli"]
    at_d, zt_d, xh_d, h_d = aps["at_d"], aps["zt_d"], aps["xh_d"], aps["h_d"]
    wf_d, wu_d, wcx_d, wcrh_d = (
        aps["wf_d"], aps["wu_d"], aps["wcx_d"], aps["wcrh_d"])
    bf_d, bu_d, bc_d, id_d, out_d = (
        aps["bf_d"], aps["bu_d"], aps["bc_d"], aps["id_d"], aps["out_d"])

    SIG = mybir.ActivationFunctionType.Sigmoid
    TANH = mybir.ActivationFunctionType.Tanh

    with (
        tc.tile_pool(name="const", bufs=1) as cpool,       # persistent small tiles
        tc.tile_pool(name="perst", bufs=1) as ppool,       # persistent activations
        tc.tile_pool(name="nmrot", bufs=2) as nmpool,      # rotating node-major tensors
        tc.tile_pool(name="cmrot", bufs=4) as cmpool,      # rotating channel-major tensors
        tc.tile_pool(name="slab", bufs=5) as slpool,       # adj slabs
        tc.tile_pool(name="psum", bufs=8, space="PSUM") as pspool,
        tc.tile_pool(name="stage", bufs=2) as stpool,      # small staging tiles
    ):
        # ---- persistent loads ----
        idm = cpool.tile([chunk, chunk], F16, tag="idm")
        nc.sync.dma_start(out=idm[:], in_=id_d[:])
        wf_sb = [cpool.tile([C, D_H], F16, tag=f"wf{k}", name=f"wf{k}")
                 for k in range(3)]
        wu_sb = [cpool.tile([C, D_H], F16, tag=f"wu{k}", name=f"wu{k}")
                 for k in range(3)]
        wcx_sb = [cpool.tile([D_IN, D_H], F16, tag=f"wcx{k}", name=f"wcx{k}")
                  for k in range(3)]
        wcrh_sb = [cpool.tile([BH, D_H], F16, tag=f"wcrh{k}", name=f"wcrh{k}")
                   for k in range(3)]
        for k in range(3):
            nc.scalar.dma_start(out=wf_sb[k][:], in_=wf_d[k])
            nc.scalar.dma_start(out=wu_sb[k][:], in_=wu_d[k])
            nc.scalar.dma_start(out=wcx_sb[k][:], in_=wcx_d[k])
            nc.scalar.dma_start(out=wcrh_sb[k][:], in_=wcrh_d[k])
        bf_sb = cpool.tile([BH, 1], F32, tag="bf")
        nc.sync.dma_start(out=bf_sb[:], in_=bf_d[:])
        bu_sb = cpool.tile([BH, 1], F32, tag="bu")
        nc.sync.dma_start(out=bu_sb[:], in_=bu_d[:])
        bc_sb = cpool.tile([BH, 1], F32, tag="bc")
        nc.sync.dma_start(out=bc_sb[:], in_=bc_d[:])

        # node-major [x;h]: one tile, chunk j occupies cols [j*BC, (j+1)*BC)
        # (rotating pool: ztT is dead after the first diffusion, z2T reuses it)
        ztT = nmpool.tile([chunk, nch * BC], F16, tag="nm", name="ztT")
        nc.sync.dma_start(
            out=ztT[:, :].rearrange("p (j f) -> p j f", j=nch),
            in_=zt_d[:, :].rearrange("(j p) f -> p j f", p=chunk))

        xh_sb = [ppool.tile([C, npad], F16, tag=f"xh{b}", name=f"xh{b}")
                 for b in range(B_LOC)]
        for b in range(B_LOC):
            nc.scalar.dma_start(out=xh_sb[b][:], in_=xh_d[b])
        # batch-stacked [b0 rows 0:64 | b1 rows 64:128]
        h_st = ppool.tile([BH, npad], F16, tag="h_st")
        for b in range(B_LOC):
            nc.scalar.dma_start(out=h_st[b * D_H:(b + 1) * D_H, :], in_=h_d[b])
        u_st = ppool.tile([BH, npad], F16, tag="u_st")
        rh_st = ppool.tile([BH, npad], F16, tag="rh_st")

        # ---- helpers ----
        def load_slab(g, jB):
            # two triggers per slab, one per HWDGE ring (SP + ACT), so both
            # trigger queues and transfer paths run in parallel
            slab = slpool.tile([chunk, jb * mg * chunk], F16, tag="slab",
                               name="slab")
            h1 = jb // 2
            eng2 = nc.scalar
            nc.sync.dma_start(
                out=slab[:, 0:h1 * mg * chunk].rearrange(
                    "p (j m) -> p j m", j=h1),
                in_=at_d[g, :, jB * jb: jB * jb + h1, :])
            eng2.dma_start(
                out=slab[:, h1 * mg * chunk:].rearrange(
                    "p (j m) -> p j m", j=jb - h1),
                in_=at_d[g, :, jB * jb + h1:(jB + 1) * jb, :])
            return slab

        def diffusion_sa(src, dst):
            """dst = A @ src, node-major -> node-major (adj stationary)."""
            for g in range(ngrp):
                pss = [pspool.tile([chunk, BC], F32, tag="ps", name=f"psd{mi}")
                       for mi in range(mg)]
                for jB in range(nch // jb):
                    slab = load_slab(g, jB)
                    for jj in range(jb):
                        j = jB * jb + jj
                        for mi in range(mg):
                            nc.tensor.matmul(
                                pss[mi][:, :],
                                lhsT=slab[:, (jj * mg + mi) * chunk:
                                          (jj * mg + mi + 1) * chunk],
                                rhs=src[:, j * BC:(j + 1) * BC],
                                start=(j == 0), stop=(j == nch - 1))
                for mi in range(mg):
                    m = g * mg + mi
                    nc.vector.tensor_copy(
                        out=dst[:, m * BC:(m + 1) * BC], in_=pss[mi][:, :])

        def diffusion_sz(src_nm, dst_bm, consumer=None):
            """dst_bm[128 bc, m] = (A @ src)^T with src (node-major [n, 128bc])
            stationary and adj moving. Optionally calls consumer(g) after the
            group band [g*nsl, (g+1)*nsl) of dst_bm is available."""
            for g in range(ngrp):
                psc = pspool.tile([BH, mg * chunk], F32, tag="ps", name="psz")
                for jB in range(nch // jb):
                    slab = load_slab(g, jB)
                    for jj in range(jb):
                        j = jB * jb + jj
                        nc.tensor.matmul(
                            psc[:, :],
                            lhsT=src_nm[:, j * chunk:(j + 1) * chunk],
                            rhs=slab[:, jj * mg * chunk:(jj + 1) * mg * chunk],
                            start=(j == 0), stop=(j == nch - 1))
                nc.vector.tensor_copy(
                    out=dst_bm[:, g * mg * chunk:(g + 1) * mg * chunk],
                    in_=psc[:, :])
                if consumer is not None:
                    consumer(g)

        def to_channel_major(src_nm):
            """node-major [chunk, nch*BC] fp16 -> per-batch channel-major [C, npad]."""
            cms = [cmpool.tile([C, npad], F16, tag="cm", name=f"cm{b}")
                   for b in range(B_LOC)]
            for b in range(B_LOC):
                for j in range(nch):
                    pt = pspool.tile([C, chunk], F16, tag="ps")
                    nc.tensor.transpose(
                        pt[:, :],
                        src_nm[:, j * BC + b * C: j * BC + (b + 1) * C],
                        idm[:, :])
                    nc.vector.tensor_copy(
                        out=cms[b][:, j * chunk:(j + 1) * chunk], in_=pt[:, :])
            return cms

        # ---- gates path: z1 = A z, z2 = A z1 ----
        z1T = nmpool.tile([chunk, nch * BC], F16, tag="nm")
        diffusion_sa(ztT, z1T)
        z2T = nmpool.tile([chunk, nch * BC], F16, tag="nm")
        diffusion_sa(z1T, z2T)

        z1cm = to_channel_major(z1T)
        z2cm = to_channel_major(z2T)

        # gate convs: r and u, batch-stacked in PSUM partitions
        # (rhT: node-major r*h, filled per band inside the loop)
        rhT = ppool.tile([chunk, nch * BH], F16, tag="rhT")
        for s in range(nsli):
            sl = slice(s * nsl, (s + 1) * nsl)
            psf = pspool.tile([BH, nsl], F32, tag="ps", name="psf")
            psu = pspool.tile([BH, nsl], F32, tag="ps", name="psu")
            for b in range(B_LOC):
                rows = slice(b * D_H, (b + 1) * D_H)
                feats = (xh_sb[b][:, sl], z1cm[b][:, sl], z2cm[b][:, sl])
                for k in range(3):
                    nc.tensor.matmul(psf[rows, :], lhsT=wf_sb[k], rhs=feats[k],
                                     start=(k == 0), stop=(k == 2))
                for k in range(3):
                    nc.tensor.matmul(psu[rows, :], lhsT=wu_sb[k], rhs=feats[k],
                                     start=(k == 0), stop=(k == 2))
            rst = stpool.tile([BH, nsl], F16, tag="rst")
            nc.scalar.activation(rst[:, :], psf[:, :], SIG, bias=bf_sb[:, :])
            nc.vector.tensor_mul(out=rh_st[:, sl], in0=rst[:, :],
                                 in1=h_st[:, sl])
            nc.scalar.activation(u_st[:, sl], psu[:, :], SIG, bias=bu_sb[:, :])
            # rhT transposes for this node band, so the candidate diffusion
            # can start as soon as the band is ready
            for b in range(B_LOC):
                rows = slice(b * D_H, (b + 1) * D_H)
                for j in range(s * nsl // chunk, (s + 1) * nsl // chunk):
                    pt = pspool.tile([chunk, D_H], F16, tag="ps", name="ptr")
                    nc.tensor.transpose(
                        pt[:, :], rh_st[rows, j * chunk:(j + 1) * chunk],
                        idm[rows, rows])
                    nc.vector.tensor_copy(
                        out=rhT[:, j * BH + b * D_H: j * BH + (b + 1) * D_H],
                        in_=pt[:, :])

        zc1_bm = ppool.tile([BH, npad], F16, tag="zc1bm")
        diffusion_sz(rhT, zc1_bm)

        zc1T = ppool.tile([chunk, nch * BH], F16, tag="zc1T")
        for j in range(nch):
            pt = pspool.tile([chunk, chunk], F16, tag="ps")
            nc.tensor.transpose(pt[:, :],
                                zc1_bm[:, j * chunk:(j + 1) * chunk], idm[:, :])
            nc.vector.tensor_copy(
                out=zc1T[:, j * chunk:(j + 1) * chunk], in_=pt[:, :])

        zc2_bm = ppool.tile([BH, npad], F16, tag="zc2bm")

        def consumer(s):
            # candidate conv for node band s, then out = c + u*(h-c)
            sl = slice(s * nsl, (s + 1) * nsl)
            psc2 = pspool.tile([BH, nsl], F32, tag="ps", name="psc2")
            for b in range(B_LOC):
                rows = slice(b * D_H, (b + 1) * D_H)
                terms = ((wcx_sb[0], xh_sb[b][0:D_IN, sl]),
                         (wcx_sb[1], z1cm[b][0:D_IN, sl]),
                         (wcx_sb[2], z2cm[b][0:D_IN, sl]),
                         (wcrh_sb[0][rows, :], rh_st[rows, sl]),
                         (wcrh_sb[1][rows, :], zc1_bm[rows, sl]),
                         (wcrh_sb[2][rows, :], zc2_bm[rows, sl]))
                for k, (wt, rhs) in enumerate(terms):
                    nc.tensor.matmul(psc2[rows, :], lhsT=wt, rhs=rhs,
                                     start=(k == 0), stop=(k == len(terms) - 1))
            cst = stpool.tile([BH, nsl], F32, tag="cst")
            nc.scalar.activation(cst[:, :], psc2[:, :], TANH, bias=bc_sb[:, :])
            t1 = stpool.tile([BH, nsl], F32, tag="t1")
            nc.vector.tensor_sub(out=t1[:, :], in0=h_st[:, sl], in1=cst[:, :])
            nc.vector.tensor_mul(out=t1[:, :], in0=u_st[:, sl], in1=t1[:, :])
            ost = stpool.tile([BH, nsl], F32, tag="ost")
            nc.vector.tensor_add(out=ost[:, :], in0=cst[:, :], in1=t1[:, :])
            w = min(nsl, nn - s * nsl)
            if w > 0:
                for b in range(B_LOC):
                    nc.scalar.dma_start(
                        out=out_d[b][:, s * nsl: s * nsl + w],
                        in_=ost[b * D_H:(b + 1) * D_H, 0:w])

        diffusion_sz(zc1T, zc2_bm, consumer=consumer)


# ---- host-side driver ----
_CACHED_NC = None
TRACE = False           # set True (e.g. from test.py) to capture an NTFF profile
TRACE_DIR = None
LAST_RESULTS = None     # BassKernelResults of the most recent kernel() call


def _host_prep(x, h, adj, Wf, bf, Wu, bu, Wc, bc, npad=NP, nn=NN, mg=4):
    """Shard + cast + layout inputs for the 8 cores. Returns list of in_maps."""
    chunk = CHUNK
    nch = npad // chunk
    ngrp = nch // mg
    # adj^T zero-padded to [npad, npad], retiled partition-major per group band
    at = np.zeros((npad, npad), dtype=np.float16)
    at[:nn, :nn] = adj.T.astype(np.float16)
    at_t = np.ascontiguousarray(
        at.reshape(nch, chunk, ngrp, mg * chunk).transpose(2, 1, 0, 3))
    idm = np.eye(chunk, dtype=np.float16)

    def wsplit(W):
        WT = W.T.astype(np.float16)                            # [3C, D_H]
        return np.ascontiguousarray(WT.reshape(3, C, D_H))

    wf3, wu3, wc3 = wsplit(Wf), wsplit(Wu), wsplit(Wc)
    wcx3 = np.ascontiguousarray(wc3[:, :D_IN])                 # [3, D_IN, D_H]
    wcrh = wc3[:, D_IN:]                                       # [3, D_H, D_H]
    wcrh3 = np.ascontiguousarray(
        np.concatenate([wcrh] * B_LOC, axis=1))                # [3, BH, D_H]

    def bstack(v):
        return np.concatenate([v] * B_LOC).reshape(BH, 1).astype(np.float32)

    shared = {
        "wf": wf3, "wu": wu3, "wcx": wcx3, "wcrh": wcrh3,
        "bf": bstack(bf), "bu": bstack(bu), "bcb": bstack(bc),
        "idm": idm, "at": at_t,
    }
    xh = np.concatenate([x, h], axis=1).astype(np.float16)     # [B, C, nn]
    xh_p = np.zeros((B, C, npad), dtype=np.float16)
    xh_p[:, :, :nn] = xh
    h_p = np.zeros((B, D_H, npad), dtype=np.float16)
    h_p[:, :, :nn] = h.astype(np.float16)
    in_maps = []
    for core in range(NCORES):
        bs = slice(core * B_LOC, (core + 1) * B_LOC)
        xh_c = xh_p[bs]                                        # [B_LOC, C, npad]
        zt_c = np.ascontiguousarray(
            xh_c.transpose(2, 0, 1).reshape(npad, B_LOC * C))
        in_maps.append(dict(shared, zt=zt_c,
                            xh=np.ascontiguousarray(xh_c),
                            h=np.ascontiguousarray(h_p[bs])))
    return in_maps


def kernel(**inputs):
    global _CACHED_NC, LAST_RESULTS
    inputs = {k: np.asarray(v) for k, v in inputs.items()}
    if _CACHED_NC is None:
        _CACHED_NC = build_program()
    in_maps = _host_prep(**inputs)
    kw = {}
    if TRACE:
        kw = dict(trace=True, tmpdir=TRACE_DIR)
    res = run_bass_kernel_spmd(_CACHED_NC, in_maps,
                               core_ids=list(range(NCORES)), **kw)
    LAST_RESULTS = res
    outs = [res.results[i]["out"] for i in range(NCORES)]
    return np.concatenate(outs, axis=0).astype(np.float32)


if __name__ == "__main__":
    rng = np.random.default_rng(0)
    ins = {
        "x": rng.standard_normal((B, D_IN, NN), dtype=np.float32),
        "h": rng.standard_normal((B, D_H, NN), dtype=np.float32),
        "adj": rng.random((NN, NN), dtype=np.float32) / NN,
        "Wf": rng.standard_normal((D_H, 3 * C), dtype=np.float32) * 0.05,
        "Wu": rng.standard_normal((D_H, 3 * C), dtype=np.float32) * 0.05,
        "Wc": rng.standard_normal((D_H, 3 * C), dtype=np.float32) * 0.05,
        "bf": rng.standard_normal(D_H).astype(np.float32) * 0.05,
        "bu": rng.standard_normal(D_H).astype(np.float32) * 0.05,
        "bc": rng.standard_normal(D_H).astype(np.float32) * 0.05,
    }
    out = kernel(**ins)
    print(out.shape, out.dtype)



# revision 7
# speedup vs baseline: 1.5057x; 1.5057x over previous
"""GCGRU cell (order-2 graph diffusion GRU) Trainium2 Bass kernel.

Strategy: data-parallel over batch (B=16 -> 2 batches per core x 8 cores).
The 4000x4000 adjacency is kept RESIDENT in SBUF as fp8 (adjT scaled by 2^11
into e4m3 range, chunk-major [128, 32 chunks, 4000]), so HBM sees it exactly
once per core (~16MB) instead of once per diffusion stage (4x33MB fp16).

All four graph diffusions run as fp8 DoubleRow matmuls (K=256 per chunk pair)
with the activations as the PE-stationary operand and adjacency slabs moving
512 output columns at a time: z1=(A z), z2=(A z1) for the gates, zc1=(A rh),
zc2=(A zc1) for the candidate. Diffused features carry the 2^11 adjacency
scale in fp16/fp8; conv weights are pre-scaled by 2^-11 host-side, and the
second-order stages descale their PSUM copy by 2^-11 on ScalarE.

Chained diffusions need node-major stationaries, so each stage's PSUM bands
are PE-transposed (128-chunk pieces) and cast to fp8 on DVE. Gate and
candidate 1x1 convs consume channel-major copies (fp16 for the dominant
direct terms, fp8 for the small diffused terms), fused per 512-node band:
sigmoid/tanh on ScalarE, r*h / u*h+(1-u)*c combines on DVE, outputs DMA'd
per band. All input casts/layout transforms are done on host in kernel().
"""

import numpy as np
import ml_dtypes

import concourse.bass as bass
from concourse import bacc
import concourse.mybir as mybir
import concourse.tile as tile
from concourse.bass_utils import run_bass_kernel_spmd

# problem constants
B, D_IN, D_H, NN = 16, 32, 64, 4000
NCORES = 8
B_LOC = B // NCORES          # batches per core
C = D_IN + D_H               # 96 channels into each gate conv
BC = B_LOC * C               # 192 batch-channel columns (b-major)
BH = B_LOC * D_H             # 128 stacked batch-hidden rows
NP = 4096                    # node contraction dim padded to 32 chunks
NCH = NP // 128              # 32 node chunks
NJP = NCH // 2               # 16 DoubleRow chunk pairs
SC = 2048.0                  # adjacency pre-scale (2^11) into fp8 range
BAND = 512                   # node band width for PSUM groups / convs
NBAND = (NN + BAND - 1) // BAND   # 8 bands, last is 416 wide

F8 = mybir.dt.float8e4
F16 = mybir.dt.float16
F32 = mybir.dt.float32
DR = mybir.MatmulPerfMode.DoubleRow
NP8 = ml_dtypes.float8_e4m3


def band_w(bi):
    return min(BAND, NN - bi * BAND)


def build_program():
    nc = bacc.Bacc("TRN2", target_bir_lowering=False, debug=False)

    # ---- DRAM I/O (all host-prepped layouts) ----
    # at[g][p, j, m] = adjT[(4g+j)*128+p, m] * SC, fp8, m < 4000
    at_d = nc.dram_tensor("at", [8, 128, 4, NN], F8, kind="ExternalInput").ap()
    # zt[p, j, c] = concat(x,h)[b, ch, j*128+p] fp8, c = b*96+ch
    zt_d = nc.dram_tensor("zt", [128, NCH, BC], F8, kind="ExternalInput").ap()
    xal_d = nc.dram_tensor("xal", [128, NP], F16, kind="ExternalInput").ap()
    hal_d = nc.dram_tensor("hal", [128, NP], F16, kind="ExternalInput").ap()
    wfu_d = nc.dram_tensor("wfu", [3, C, 2 * D_H], F16, kind="ExternalInput").ap()
    wg0x_d = nc.dram_tensor("wg0x", [128, 2 * D_H], F16, kind="ExternalInput").ap()
    wg0h_d = nc.dram_tensor("wg0h", [128, 2 * D_H], F16, kind="ExternalInput").ap()
    wcx0_d = nc.dram_tensor("wcx0", [128, D_H], F16, kind="ExternalInput").ap()
    wcxz_d = nc.dram_tensor("wcxz", [2 * D_H, D_H], F16,
                            kind="ExternalInput").ap()
    wch_d = nc.dram_tensor("wch", [2 * D_H, D_H], F16, kind="ExternalInput").ap()
    wch2_d = nc.dram_tensor("wch2", [D_H, D_H], F16, kind="ExternalInput").ap()
    bfu_d = nc.dram_tensor("bfu", [2 * D_H, 1], F32, kind="ExternalInput").ap()
    bcb_d = nc.dram_tensor("bcb", [2 * D_H, 1], F32, kind="ExternalInput").ap()
    id_d = nc.dram_tensor("idm", [128, 128], F16, kind="ExternalInput").ap()
    out_d = nc.dram_tensor("out", [B_LOC, D_H, NN], F32,
                           kind="ExternalOutput").ap()

    with tile.TileContext(nc) as tc:
        _body(tc, at_d, zt_d, xal_d, hal_d, wfu_d, wg0x_d, wg0h_d,
              wcx0_d, wcxz_d, wch_d, wch2_d, bfu_d, bcb_d, id_d, out_d)
    nc.compile()
    return nc


def _body(tc, at_d, zt_d, xal_d, hal_d, wfu_d, wg0x_d, wg0h_d,
          wcx0_d, wcxz_d, wch_d, wch2_d, bfu_d, bcb_d, id_d, out_d):
    nc = tc.nc
    SIG = mybir.ActivationFunctionType.Sigmoid
    TANH = mybir.ActivationFunctionType.Tanh
    COPY = mybir.ActivationFunctionType.Copy

    with (
        tc.tile_pool(name="const", bufs=1) as cpool,     # weights/bias/idm/at
        tc.tile_pool(name="ztp", bufs=2) as ztpool,      # zt8 <-> z1T8 reuse
        tc.tile_pool(name="act8", bufs=1) as a8pool,     # persistent fp8 acts
        tc.tile_pool(name="perst", bufs=1) as ppool,     # persistent fp16 acts
        tc.tile_pool(name="bnd", bufs=8) as bpool,       # band staging tiles
        tc.tile_pool(name="stg", bufs=3) as stpool,      # combine staging
        tc.tile_pool(name="psmm", bufs=6, space="PSUM") as mmpool,
        tc.tile_pool(name="psx", bufs=2, space="PSUM") as txpool,
    ):
        # ---- persistent loads ----
        idm = cpool.tile([128, 128], F16, tag="idm")
        nc.sync.dma_start(out=idm[:], in_=id_d[:])
        wfu = [cpool.tile([C, 2 * D_H], F16, tag=f"wfu{k}", name=f"wfu{k}")
               for k in range(3)]
        for k in range(3):
            nc.scalar.dma_start(out=wfu[k][:], in_=wfu_d[k])
        wg0x = cpool.tile([128, 2 * D_H], F16, tag="wg0x")
        nc.sync.dma_start(out=wg0x[:], in_=wg0x_d[:])
        wg0h = cpool.tile([128, 2 * D_H], F16, tag="wg0h")
        nc.sync.dma_start(out=wg0h[:], in_=wg0h_d[:])
        wcx0 = cpool.tile([128, D_H], F16, tag="wcx0")
        nc.sync.dma_start(out=wcx0[:], in_=wcx0_d[:])
        wcxz = cpool.tile([2 * D_H, D_H], F16, tag="wcxz")
        nc.sync.dma_start(out=wcxz[:], in_=wcxz_d[:])
        wch = cpool.tile([2 * D_H, D_H], F16, tag="wch")
        nc.sync.dma_start(out=wch[:], in_=wch_d[:])
        wch2 = cpool.tile([D_H, D_H], F16, tag="wch2")
        nc.sync.dma_start(out=wch2[:], in_=wch2_d[:])
        bfu = cpool.tile([2 * D_H, 1], F32, tag="bfu")
        nc.sync.dma_start(out=bfu[:], in_=bfu_d[:])
        bcb = cpool.tile([2 * D_H, 1], F32, tag="bcb")
        nc.sync.dma_start(out=bcb[:], in_=bcb_d[:])
        xall = ppool.tile([128, NP], F16, tag="xall")
        nc.scalar.dma_start(out=xall[:], in_=xal_d[:])
        hall = ppool.tile([128, NP], F16, tag="hall")
        nc.scalar.dma_start(out=hall[:], in_=hal_d[:])

        zt8 = ztpool.tile([128, NCH * BC], F8, tag="ztn", name="zt8")
        nc.sync.dma_start(
            out=zt8[:, :].rearrange("p (j c) -> p j c", j=NCH), in_=zt_d[:])

        # resident adjacency, 8 slab DMAs alternating HWDGE rings
        at8 = cpool.tile([128, NCH * NN], F8, tag="at8")
        at3 = at8[:, :].rearrange("p (j m) -> p j m", j=NCH)
        for g in range(8):
            eng = nc.sync if g % 2 == 0 else nc.scalar
            eng.dma_start(out=at3[:, 4 * g:4 * g + 4, :], in_=at_d[g])

        # persistent activations
        u_st = ppool.tile([BH, NP], F16, tag="u_st")     # u, rows b*64
        z1cm8 = [a8pool.tile([C, NP], F8, tag=f"z1cm{b}", name=f"z1cm{b}")
                 for b in range(B_LOC)]
        # candidate diffused-x features, rows [b0z1x|b0z2x|b1z1x|b1z2x]
        candx = a8pool.tile([128, NP], F8, tag="candx")
        rhz = [a8pool.tile([2 * D_H, NP], F8, tag=f"rhz{b}", name=f"rhz{b}")
               for b in range(B_LOC)]                    # [rh|zc1] rows
        rhT8 = a8pool.tile([128, NCH * BH], F8, tag="rhT8")
        zc1T8 = a8pool.tile([128, NCH * BH], F8, tag="zc1T8")
        z1T8 = ztpool.tile([128, NCH * BC], F8, tag="ztn", name="z1T8")

        zt3 = zt8[:, :].rearrange("p (j c) -> p j c", j=NCH)
        z1T3 = z1T8[:, :].rearrange("p (j c) -> p j c", j=NCH)
        rhT3 = rhT8[:, :].rearrange("p (j c) -> p j c", j=NCH)
        zc1T3 = zc1T8[:, :].rearrange("p (j c) -> p j c", j=NCH)

        # zero the padded node rows (4000:4096 live in chunk 31) of the
        # on-chip-built stationaries so later contractions see zeros; the
        # band transposes overwrite rows 0:32 with real data afterwards
        nc.vector.memset(z1T3[:, 31, :], 0.0)
        nc.vector.memset(rhT3[:, 31, :], 0.0)
        nc.vector.memset(zc1T3[:, 31, :], 0.0)

        def diffuse_bc(src3, bands, psname):
            """DoubleRow diffusion, [*,*,BC]-layout stationary; returns
            per-(band, batch) psum tiles [C, w] accumulated over all nodes."""
            pss = {}
            for bi in bands:
                for b in range(B_LOC):
                    pss[bi, b] = mmpool.tile([C, BAND], F32, tag="mm",
                                             name=f"{psname}_{bi}_{b}")
            for jp in range(NJP):
                for b in range(B_LOC):
                    lhsT = src3[:, 2 * jp:2 * jp + 2, b * C:(b + 1) * C]
                    for bi in bands:
                        w = band_w(bi)
                        nc.tensor.matmul(
                            pss[bi, b][:, 0:w], lhsT=lhsT,
                            rhs=at3[:, 2 * jp:2 * jp + 2,
                                    bi * BAND:bi * BAND + w],
                            start=(jp == 0), stop=(jp == NJP - 1),
                            perf_mode=DR)
            return pss

        def diffuse_bh(src3, bands, psname):
            """DoubleRow diffusion, [*,*,BH]-layout stationary (both batches
            in one tile); returns per-band psum tiles [BH, w]."""
            pss = {}
            for bi in bands:
                pss[bi] = mmpool.tile([BH, BAND], F32, tag="mm",
                                      name=f"{psname}_{bi}")
            for jp in range(NJP):
                lhsT = src3[:, 2 * jp:2 * jp + 2, :]
                for bi in bands:
                    w = band_w(bi)
                    nc.tensor.matmul(
                        pss[bi][:, 0:w], lhsT=lhsT,
                        rhs=at3[:, 2 * jp:2 * jp + 2, bi * BAND:bi * BAND + w],
                        start=(jp == 0), stop=(jp == NJP - 1), perf_mode=DR)
            return pss

        def transpose_band(srcT, r0, rows, bi, dst3, dcol0, dcols):
            """PE-transpose band staging srcT[r0:r0+rows, 0:w] (f16) into fp8
            node-major dst3[:, j, dcol0:dcol0+dcols] chunk pieces."""
            w = band_w(bi)
            m0 = bi * BAND
            rsl = slice(r0, r0 + rows)
            for j in range(m0 // 128, (m0 + w + 127) // 128):
                off = j * 128 - m0
                wj = min(128, w - off)
                pt = txpool.tile([128, rows], F16, tag="tx", name=f"tp{j}")
                nc.tensor.transpose(pt[0:wj, :], srcT[rsl, off:off + wj],
                                    idm[rsl, rsl])
                nc.vector.tensor_copy(
                    out=dst3[0:wj, j, dcol0:dcol0 + dcols], in_=pt[0:wj, :])

        # ================= stage 1: z1 = (A*SC) z =================
        for q in range(NBAND // 2):
            bands = (2 * q, 2 * q + 1)
            pss = diffuse_bc(zt3, bands, "ps1")
            for bi in bands:
                w = band_w(bi)
                msl = slice(bi * BAND, bi * BAND + w)
                for b in range(B_LOC):
                    ps = pss[bi, b]
                    z1b = bpool.tile([C, BAND], F16, tag="bnd", name="z1b")
                    nc.scalar.activation(z1b[:, 0:w], ps[:, 0:w], COPY)
                    nc.vector.tensor_copy(out=z1cm8[b][:, msl], in_=ps[:, 0:w])
                    nc.vector.tensor_copy(
                        out=candx[b * D_H:b * D_H + D_IN, msl],
                        in_=ps[0:D_IN, 0:w])
                    transpose_band(z1b, 0, C, bi, z1T3, b * C, C)

        # ============ stage 2: z2 = (A*SC) z1s, gates, rh ============
        for q in range(NBAND // 2):
            bands = (2 * q, 2 * q + 1)
            pss = diffuse_bc(z1T3, bands, "ps2")
            for bi in bands:
                w = band_w(bi)
                msl = slice(bi * BAND, bi * BAND + w)
                rst = bpool.tile([BH, BAND], F16, tag="bnd", name="rst")
                for b in range(B_LOC):
                    ps = pss[bi, b]
                    # z2s = 2^11 z2 (psum holds 2^22 z2)
                    z2b = bpool.tile([C, BAND], F16, tag="bnd", name="z2b")
                    nc.scalar.activation(z2b[:, 0:w], ps[:, 0:w], COPY,
                                         scale=1.0 / SC)
                    nc.vector.tensor_scalar_mul(
                        candx[b * D_H + D_IN:b * D_H + 2 * D_IN, msl],
                        ps[0:D_IN, 0:w], 1.0 / SC)
                    # gate conv: r|u preact = Wfu0 z + Wfu1' z1s + Wfu2' z2s
                    psg = mmpool.tile([2 * D_H, BAND], F32, tag="mm",
                                      name="psg")
                    gt = ((wg0x[b * D_H:b * D_H + D_IN, :],
                           xall[b * D_H:b * D_H + D_IN, msl]),
                          (wg0h[b * D_H:(b + 1) * D_H, :],
                           hall[b * D_H:(b + 1) * D_H, msl]),
                          (wfu[1], z1cm8[b][:, msl]),
                          (wfu[2], z2b[:, 0:w]))
                    for k, (wt, rhs) in enumerate(gt):
                        nc.tensor.matmul(psg[:, 0:w], lhsT=wt, rhs=rhs,
                                         start=(k == 0),
                                         stop=(k == len(gt) - 1))
                    rows = slice(b * D_H, (b + 1) * D_H)
                    nc.scalar.activation(rst[rows, 0:w], psg[0:D_H, 0:w],
                                         SIG, bias=bfu[0:D_H, :])
                    nc.scalar.activation(u_st[rows, msl],
                                         psg[D_H:2 * D_H, 0:w], SIG,
                                         bias=bfu[D_H:2 * D_H, :])
                if True:  # both batches' r now staged batch-stacked in rst
                    rhb = bpool.tile([BH, BAND], F16, tag="bnd", name="rhb")
                    nc.vector.tensor_mul(out=rhb[:, 0:w], in0=rst[:, 0:w],
                                         in1=hall[:, msl])
                    for b in range(B_LOC):
                        rows = slice(b * D_H, (b + 1) * D_H)
                        nc.vector.tensor_copy(out=rhz[b][0:D_H, msl],
                                              in_=rhb[rows, 0:w])
                        transpose_band(rhb, b * D_H, D_H, bi, rhT3,
                                       b * D_H, D_H)

        # ================= stage 3: zc1 = (A*SC) rh =================
        for hh in range(NBAND // 4):
            bands = tuple(range(4 * hh, 4 * hh + 4))
            pss = diffuse_bh(rhT3, bands, "ps3")
            for bi in bands:
                w = band_w(bi)
                msl = slice(bi * BAND, bi * BAND + w)
                for b in range(B_LOC):
                    rows = slice(b * D_H, (b + 1) * D_H)
                    zc1b = bpool.tile([D_H, BAND], F16, tag="bnd",
                                      name="zc1b")
                    nc.scalar.activation(zc1b[:, 0:w], pss[bi][rows, 0:w],
                                         COPY)
                    nc.vector.tensor_copy(out=rhz[b][D_H:2 * D_H, msl],
                                          in_=pss[bi][rows, 0:w])
                    transpose_band(zc1b, 0, D_H, bi, zc1T3, b * D_H, D_H)

        # ================ stage 4: zc2, candidate, output ================
        for hh in range(NBAND // 4):
            bands = tuple(range(4 * hh, 4 * hh + 4))
            pss = diffuse_bh(zc1T3, bands, "ps4")
            for bi in bands:
                w = band_w(bi)
                msl = slice(bi * BAND, bi * BAND + w)
                psc = mmpool.tile([BH, BAND], F32, tag="mm", name="psc")
                for b in range(B_LOC):
                    rows = slice(b * D_H, (b + 1) * D_H)
                    zc2b = bpool.tile([D_H, BAND], F16, tag="bnd",
                                      name="zc2b")
                    nc.scalar.activation(zc2b[:, 0:w], pss[bi][rows, 0:w],
                                         COPY, scale=1.0 / SC)
                    terms = ((wcx0[b * D_H:b * D_H + D_IN, :],
                              xall[b * D_H:b * D_H + D_IN, msl]),
                             (wcxz[b * D_H:(b + 1) * D_H, :],
                              candx[b * D_H:(b + 1) * D_H, msl]),
                             (wch, rhz[b][:, msl]),
                             (wch2, zc2b[:, 0:w]))
                    for k, (wt, rhs) in enumerate(terms):
                        nc.tensor.matmul(psc[rows, 0:w], lhsT=wt, rhs=rhs,
                                         start=(k == 0),
                                         stop=(k == len(terms) - 1))
                cst = bpool.tile([BH, BAND], F16, tag="bnd", name="cst")
                nc.scalar.activation(cst[:, 0:w], psc[:, 0:w], TANH,
                                     bias=bcb[:, :])
                t1 = stpool.tile([BH, BAND], F32, tag="cmb", name="t1")
                nc.vector.tensor_sub(out=t1[:, 0:w], in0=hall[:, msl],
                                     in1=cst[:, 0:w])
                nc.vector.tensor_mul(out=t1[:, 0:w], in0=u_st[:, msl],
                                     in1=t1[:, 0:w])
                ost = stpool.tile([BH, BAND], F32, tag="cmb", name="ost")
                nc.vector.tensor_add(out=ost[:, 0:w], in0=cst[:, 0:w],
                                     in1=t1[:, 0:w])
                for b in range(B_LOC):
                    nc.scalar.dma_start(
                        out=out_d[b][:, msl],
                        in_=ost[b * D_H:(b + 1) * D_H, 0:w])


# ---- host-side driver ----
_CACHED_NC = None
TRACE = False
TRACE_DIR = None
LAST_RESULTS = None


def _f8(a):
    return np.clip(a, -240.0, 240.0).astype(NP8)


def _host_prep(x, h, adj, Wf, bf, Wu, bu, Wc, bc):
    # adjacency: transpose, scale, pad contraction rows to 4096, fp8,
    # chunk-major groups of 4
    at = np.zeros((NP, NN), dtype=np.float32)
    at[:NN, :] = adj.T * SC
    at8 = _f8(at).reshape(8, 4, 128, NN).transpose(0, 2, 1, 3)
    at8 = np.ascontiguousarray(at8)                       # [8, 128, 4, 4000]
    idm = np.eye(128, dtype=np.float16)

    def wsplit(W, s):  # [D_H, 3C] -> [3, C, D_H] with per-order scales
        WT = np.ascontiguousarray(W.T.reshape(3, C, D_H)).astype(np.float32)
        return WT * np.asarray(s, np.float32)[:, None, None]

    wf3 = wsplit(Wf, [1, 1 / SC, 1 / SC])
    wu3 = wsplit(Wu, [1, 1 / SC, 1 / SC])
    wc3 = wsplit(Wc, [1, 1 / SC, 1 / SC])
    # gate weights packed [96, f64|u64] per order
    wfu = np.concatenate([wf3, wu3], axis=2).astype(np.float16)  # [3, 96, 128]
    # direct-term gate weights split x/h, duplicated per batch row-block so
    # matmul stationary/moving base partitions match (b0 rows 0:*, b1 at 64:*)
    wg0x = np.zeros((128, 2 * D_H), dtype=np.float16)
    wg0x[0:D_IN] = wg0x[D_H:D_H + D_IN] = wfu[0, :D_IN]
    wg0h = np.zeros((128, 2 * D_H), dtype=np.float16)
    wg0h[0:D_H] = wg0h[D_H:2 * D_H] = wfu[0, D_IN:]
    wcx0 = np.zeros((128, D_H), dtype=np.float16)
    wcx0[0:D_IN] = wcx0[D_H:D_H + D_IN] = wc3[0, :D_IN].astype(np.float16)
    # [z1x|z2x] weights, duplicated per batch row-block for base matching
    wcxz1 = np.concatenate([wc3[1, :D_IN], wc3[2, :D_IN]])       # [64, 64]
    wcxz = np.concatenate([wcxz1, wcxz1]).astype(np.float16)     # [128, 64]
    wch = np.concatenate([wc3[0, D_IN:], wc3[1, D_IN:]]).astype(np.float16)
    wch2 = wc3[2, D_IN:].astype(np.float16)               # [64, 64]
    bfu = np.concatenate([bf, bu]).reshape(2 * D_H, 1).astype(np.float32)
    bcb = np.concatenate([bc, bc]).reshape(2 * D_H, 1).astype(np.float32)

    shared = {"at": at8, "wfu": wfu, "wg0x": wg0x, "wg0h": wg0h,
              "wcx0": wcx0, "wcxz": wcxz, "wch": wch, "wch2": wch2,
              "bfu": bfu, "bcb": bcb, "idm": idm}

    z = np.concatenate([x, h], axis=1)                    # [B, 96, 4000]
    zp = np.zeros((B, C, NP), dtype=np.float32)
    zp[:, :, :NN] = z
    in_maps = []
    for core in range(NCORES):
        bs = slice(core * B_LOC, (core + 1) * B_LOC)
        zc = zp[bs]                                       # [2, 96, 4096]
        # node-major fp8 [128, 32, 192]
        zt = _f8(zc.transpose(2, 0, 1).reshape(NCH, 128, BC)
                 .transpose(1, 0, 2))
        # batch-stacked x (rows 0:32, 64:96) and h (rows 0:64, 64:128)
        xal = np.zeros((128, NP), dtype=np.float16)
        hal = np.zeros((128, NP), dtype=np.float16)
        for b in range(B_LOC):
            xal[b * D_H:b * D_H + D_IN] = zc[b, :D_IN]
            hal[b * D_H:(b + 1) * D_H] = zc[b, D_IN:]
        in_maps.append(dict(shared, zt=np.ascontiguousarray(zt),
                            xal=xal, hal=hal))
    return in_maps


def kernel(**inputs):
    global _CACHED_NC, LAST_RESULTS
    inputs = {k: np.asarray(v) for k, v in inputs.items()}
    if _CACHED_NC is None:
        _CACHED_NC = build_program()
    in_maps = _host_prep(**inputs)
    kw = {}
    if TRACE:
        kw = dict(trace=True, tmpdir=TRACE_DIR)
    res = run_bass_kernel_spmd(_CACHED_NC, in_maps,
                               core_ids=list(range(NCORES)), **kw)
    LAST_RESULTS = res
    outs = [res.results[i]["out"] for i in range(NCORES)]
    return np.concatenate(outs, axis=0).astype(np.float32)


if __name__ == "__main__":
    rng = np.random.default_rng(0)
    ins = {
        "x": rng.standard_normal((B, D_IN, NN), dtype=np.float32),
        "h": rng.standard_normal((B, D_H, NN), dtype=np.float32),
        "adj": rng.random((NN, NN), dtype=np.float32) / NN,
        "Wf": rng.standard_normal((D_H, 3 * C), dtype=np.float32) * 0.05,
        "Wu": rng.standard_normal((D_H, 3 * C), dtype=np.float32) * 0.05,
        "Wc": rng.standard_normal((D_H, 3 * C), dtype=np.float32) * 0.05,
        "bf": rng.standard_normal(D_H).astype(np.float32) * 0.05,
        "bu": rng.standard_normal(D_H).astype(np.float32) * 0.05,
        "bc": rng.standard_normal(D_H).astype(np.float32) * 0.05,
    }
    out = kernel(**ins)
    print(out.shape, out.dtype)
